# revision 3
# baseline (speedup 1.0000x reference)
"""BoundaryLoss kernel for Trainium2 NeuronCores (axon-tunneled).

Computes mean |pred_dist - target_dist| where *_dist are sums of per-class
exact Euclidean distance transforms of the argmax(pred) / target masks.

End-to-end wall time is dominated by the axon dispatch round trip
(~70 ms floor) plus ~12 ms/MB of input transfer, so the design minimizes
host->device bytes and per-call dispatch overhead:

  - argmax(pred) is computed on host; the two class masks are packed into
    ONE uint8 tensor (pred_mask << 4 | target) -- 64 KB per core instead
    of the 1.3 MB of raw fp32 logits.
  - 4 cores, one full image each (no halo rows, both masks on the same
    core so |pred_dist - target_dist| reduces locally to [128,1]).
  - the jitted shard_map dispatcher is built ONCE per (R-bucket) and
    cached; per-call cost is transfer + execute only.

EDT algorithm per (mask, class, image):
  pass 1 (along W): exact nearest-set-pixel row distances via two
    min-plus scans  state = min(state+1, f)  (forward + backward).
  pass 2 (along H): d^2(x) = min_k (dr[x+k]^2 + k^2) windowed to |k| <= R,
    where R is a sound data-derived bound (max row distance plus the max
    empty-row gap), rounded up to a bucket so one NEFF is reused.
"""

import numpy as np

import jax
from jax.sharding import Mesh, PartitionSpec
from jax.experimental.shard_map import shard_map

import concourse.bass as bass
import concourse.bacc as bacc
import concourse.mybir as mybir
from concourse.tile import TileContext
from concourse import bass2jax as _b2j

B, C, H, W = 4, 4, 256, 256
N_CORES = 4
LARGEF = 1.0e6  # pseudo-infinity seed for pass-1 scans (pre-square space)
INF = 1 << 20

F32 = mybir.dt.float32
I16 = mybir.dt.int16
U8 = mybir.dt.uint8
Alu = mybir.AluOpType
Act = mybir.ActivationFunctionType

# R buckets (multiples of 16 keep DMA-transpose offsets 32B-aligned).
# <=112 keeps row dists <= 127 so squares fit comfortably in int16.
_BUCKETS_I16 = (32, 48, 64, 80, 96, 112)
_BUCKETS_F32 = (176, 256, 368)


# ---------------------------------------------------------------- host plan

def _argmax4(pred):
    """First-wins argmax over axis 1 of [B,4,H,W], as uint8."""
    p0, p1, p2, p3 = pred[:, 0], pred[:, 1], pred[:, 2], pred[:, 3]
    m01 = np.maximum(p0, p1)
    m23 = np.maximum(p2, p3)
    i01 = (p1 > p0).astype(np.uint8)
    i23 = np.where(p3 > p2, np.uint8(3), np.uint8(2))
    return np.where(m23 > m01, i23, i01)


def _plan_fast(pm, tg):
    """Window radius R and per-(image, slab) presence flags.

    Scans run along axis 0 of W-transposed arrays so numpy's
    minimum.accumulate vectorizes (SIMD across columns).
    """
    pmT = np.ascontiguousarray(pm.reshape(-1, W).T)  # [W, B*H]
    tgT = np.ascontiguousarray(tg.reshape(-1, W).T)
    BH = B * H
    bsT = np.empty((W, 6 * BH), np.bool_)
    for i, c in enumerate((1, 2, 3)):
        np.equal(pmT, c, out=bsT[:, i * BH:(i + 1) * BH])
        np.equal(tgT, c, out=bsT[:, (3 + i) * BH:(4 + i) * BH])
    idx = np.arange(W, dtype=np.float32)[:, None]
    d = np.where(bsT, np.float32(0), np.float32(INF))
    fwd = np.minimum.accumulate(d - idx, axis=0) + idx
    bwd = np.minimum.accumulate((d + idx)[::-1], axis=0)[::-1] - idx
    dr = np.minimum(fwd, bwd)  # [W, 6*BH]
    drf = np.where(dr < INF // 2, dr, np.float32(-1))
    r1 = drf.reshape(W, 6, BH).max(axis=(0, 2))  # [6], -1 if slab empty

    # row (b,h) of slab s nonempty <=> its first row-dist is finite
    rows_any = (dr[0] < INF // 2).reshape(6, B, H)
    present = rows_any.any(axis=2)  # [6, B]
    raT = np.ascontiguousarray(rows_any.reshape(-1, H).T)  # [H, 6*B]
    idxH = np.arange(H, dtype=np.float32)[:, None]
    dH = np.where(raT, np.float32(0), np.float32(INF))
    fH = np.minimum.accumulate(dH - idxH, axis=0) + idxH
    bH = np.minimum.accumulate((dH + idxH)[::-1], axis=0)[::-1] - idxH
    drH = np.minimum(fH, bH)  # [H, 6*B]
    vg = np.where(present.reshape(1, -1), drH, np.float32(0)).max(axis=0)
    vg = vg.reshape(6, B).max(axis=1)  # [6]

    R = int(max(1, np.minimum(r1 + vg, 361).max()))
    flags = present.T.astype(np.float32)  # [B, 6], slab = mi*3 + (c-1)
    return R, flags


def _bucket(R):
    for b in _BUCKETS_I16 + _BUCKETS_F32:
        if R <= b:
            return b
    return _BUCKETS_F32[-1]


# ---------------------------------------------------------------- device side

def _build(R, use_i16, iters=1):
    P = 2 * R + 256  # padded column length for pass 2
    capv = 127.0 if use_i16 else 400.0
    padv = 30000 if use_i16 else 1.0e9
    DT = I16 if use_i16 else F32

    nc = bacc.Bacc(None, target_bir_lowering=False)
    maskP = nc.dram_tensor("maskP", [H, W], U8, kind="ExternalInput")
    flagsI = nc.dram_tensor("flags", [128, 6], F32, kind="ExternalInput")
    out = nc.dram_tensor("out", [128, 1], F32, kind="ExternalOutput")

    with TileContext(nc) as tc:
        with (
            tc.tile_pool(name="const", bufs=1) as constp,
            tc.tile_pool(name="io", bufs=2) as iop,
            tc.tile_pool(name="p1", bufs=2) as p1p,
            tc.tile_pool(name="h2", bufs=1) as h2p,
            tc.tile_pool(name="fin", bufs=1) as finp,
        ):
            def _body():
                flagst = constp.tile([128, 6], F32)
                nc.gpsimd.dma_start(flagst[:], flagsI[:])
                ones = constp.tile([128, W], F32)
                nc.vector.memset(ones[:], 1.0)

                # W-transposed row-distance maps, 6 slabs
                # (pred c1..c3, targ c1..c3), free len P = 256 + 2R padding.
                # h2d pads (never written by the transposes) hold capv so
                # their squares read as "far" for boundary rows. h2B = h2A
                # shifted one element left (odd window offsets keep the
                # 2x_1P int16 DVE mode).
                h2d = [h2p.tile([128, 6, P], I16, name=f"h2d{w}") for w in range(2)]
                h2A = [h2p.tile([128, 6, P], DT, name=f"h2A{w}") for w in range(2)]
                if use_i16:
                    h2B = [h2p.tile([128, 6, P], DT, name=f"h2B{w}") for w in range(2)]
                accs = [h2p.tile([128, 6, 256], DT, name=f"acc{w}") for w in range(2)]
                for wc in range(2):
                    nc.vector.memset(h2d[wc][:], capv)
                    if use_i16:
                        nc.vector.memset(h2B[wc][:], padv)
                    nc.vector.memset(accs[wc][:], padv)

                # ---------------- pass 1 + transpose, per row-chunk
                for cs in (0, 128):
                    mk = iop.tile([128, W], U8, name="mk")
                    nc.gpsimd.dma_start(mk[:], maskP[cs:cs + 128])
                    pmu = p1p.tile([128, W], U8, name="pmu")
                    nc.vector.tensor_scalar(
                        pmu[:], mk[:], 4, None, op0=Alu.logical_shift_right)
                    tau = p1p.tile([128, W], U8, name="tau")
                    nc.vector.tensor_scalar(
                        tau[:], mk[:], 15, None, op0=Alu.bitwise_and)
                    pmf = p1p.tile([128, W], F32, name="pmf")
                    nc.scalar.activation(pmf[:], pmu[:], Act.Copy)
                    taf = p1p.tile([128, W], F32, name="taf")
                    nc.scalar.activation(taf[:], tau[:], Act.Copy)

                    for slab in range(6):
                        mi, c = divmod(slab, 3)
                        c += 1
                        src = taf if mi == 1 else pmf
                        f = p1p.tile([128, W], F32, name="fseed")
                        nc.vector.tensor_scalar(
                            f[:], src[:], float(c), LARGEF,
                            op0=Alu.not_equal, op1=Alu.mult)
                        a = p1p.tile([128, W], F32, name="a")
                        nc.vector.tensor_tensor_scan(
                            a[:], ones[:], f[:], LARGEF,
                            op0=Alu.add, op1=Alu.min)
                        dd = p1p.tile([128, W], F32, name="dd")
                        nc.vector.tensor_tensor_scan(
                            dd[:, ::-1], ones[:], a[:, ::-1], LARGEF,
                            op0=Alu.add, op1=Alu.min)
                        nc.vector.tensor_scalar_min(dd[:], dd[:], capv)
                        ddi = p1p.tile([128, W], I16, name="ddi")
                        nc.gpsimd.tensor_copy(ddi[:], dd[:])

                        for wc in range(2):
                            nc.sync.dma_start_transpose(
                                h2d[wc][:, slab, R + cs:R + cs + 128],
                                ddi[:, wc * 128:(wc + 1) * 128])

                # squares: h2A = h2d^2, h2B = shifted h2A
                for wc in range(2):
                    nc.scalar.activation(h2A[wc][:], h2d[wc][:], Act.Square)
                    if use_i16:
                        nc.scalar.activation(
                            h2B[wc][:, :, 0:P - 1],
                            h2d[wc][:, :, 1:P], Act.Square)

                # ---------------- pass 2: windowed parabola min-plus along H
                ks = [0]
                for k in range(1, R + 1):
                    ks += [k, -k]
                for k in ks:
                    base = R + k
                    kk = k * k
                    for wc in range(2):
                        if use_i16 and base % 2 == 1:
                            src, b0 = h2B[wc], base - 1
                        else:
                            src, b0 = h2A[wc], base
                        nc.vector.scalar_tensor_tensor(
                            accs[wc][:], src[:, :, b0:b0 + 256],
                            int(kk) if use_i16 else float(kk),
                            accs[wc][:],
                            op0=Alu.add, op1=Alu.min)

                # ---------------- sqrt, class sums, |pred-targ|, reduce
                prt = finp.tile([128, 2], F32)
                for wc in range(2):
                    sq = finp.tile([128, 6, 256], F32, name="sq")
                    for slab in range(6):
                        nc.scalar.activation(
                            sq[:, slab], accs[wc][:, slab], Act.Sqrt)
                        nc.vector.tensor_single_scalar(
                            sq[:, slab], sq[:, slab],
                            flagst[:, slab:slab + 1], op=Alu.mult)
                    sp = finp.tile([128, 256], F32, name="sp")
                    st = finp.tile([128, 256], F32, name="st")
                    nc.vector.tensor_add(sp[:], sq[:, 0], sq[:, 1])
                    nc.vector.tensor_add(sp[:], sp[:], sq[:, 2])
                    nc.vector.tensor_add(st[:], sq[:, 3], sq[:, 4])
                    nc.vector.tensor_add(st[:], st[:], sq[:, 5])
                    nc.vector.tensor_sub(sp[:], sp[:], st[:])
                    nc.vector.tensor_reduce(
                        prt[:, wc:wc + 1], sp[:], axis=mybir.AxisListType.X,
                        op=Alu.add, apply_absolute_value=True)
                total = finp.tile([128, 1], F32)
                nc.vector.tensor_add(total[:], prt[:, 0:1], prt[:, 1:2])
                nc.gpsimd.dma_start(out[:], total[:])

            if iters > 1:
                E = mybir.EngineType
                with tc.For_i(0, iters, 1, hint_engines=(
                        E.DVE, E.Activation, E.Pool, E.SP)):
                    _body()
            else:
                _body()

    nc.finalize()
    return nc


# ---------------------------------------------------------------- dispatcher

_EXEC_CACHE = {}


def _get_exec(Rb, use_i16, iters=1):
    """Build the Bass module + jitted shard_map dispatcher once per bucket."""
    key = (Rb, use_i16, iters)
    if key in _EXEC_CACHE:
        return _EXEC_CACHE[key]

    nc = _build(Rb, use_i16, iters)
    _b2j.install_neuronx_cc_hook()
    assert nc.dbg_addr is None
    part_name = nc.partition_id_tensor.name if nc.partition_id_tensor else None

    in_names, out_names, out_avals, zero_specs = [], [], [], []
    for alloc in nc.m.functions[0].allocations:
        if not isinstance(alloc, mybir.MemoryLocationSet):
            continue
        name = alloc.memorylocations[0].name
        if alloc.kind == "ExternalInput":
            if name != part_name:
                in_names.append(name)
        elif alloc.kind == "ExternalOutput":
            shape = tuple(alloc.tensor_shape)
            dtype = mybir.dt.np(alloc.dtype)
            out_names.append(name)
            out_avals.append(jax.core.ShapedArray(shape, dtype))
            zero_specs.append((shape, dtype))
    n_params = len(in_names)
    all_names = tuple(in_names) + tuple(out_names)
    if part_name is not None:
        all_names += (part_name,)

    def _body(*args):
        operands = list(args)
        if part_name is not None:
            operands.append(_b2j.partition_id_tensor())
        return tuple(_b2j._bass_exec_p.bind(
            *operands,
            out_avals=tuple(out_avals),
            in_names=all_names,
            out_names=tuple(out_names),
            lowering_input_output_aliases=(),
            sim_require_finite=True,
            sim_require_nnan=True,
            nc=nc,
        ))

    mesh = Mesh(np.asarray(jax.devices()[:N_CORES]), ("core",))
    sharded = jax.jit(
        shard_map(
            _body, mesh=mesh,
            in_specs=(PartitionSpec("core"),) * (n_params + len(out_names)),
            out_specs=(PartitionSpec("core"),) * len(out_names),
            check_rep=False,
        ),
        donate_argnums=tuple(range(n_params, n_params + len(out_names))),
        keep_unused=True,
    )

    def run(feed):
        args = [feed[n] for n in in_names]
        zeros = [np.zeros((N_CORES * s[0], *s[1:]), d) for s, d in zero_specs]
        outs = sharded(*args, *zeros)
        return {n: np.asarray(o) for n, o in zip(out_names, outs)}

    _EXEC_CACHE[key] = run
    return run


# ---------------------------------------------------------------- entry point

def kernel(pred, target):
    pred = np.asarray(pred, dtype=np.float32)
    target = np.asarray(target)
    pm = _argmax4(pred)
    tg = target.astype(np.uint8)
    R, flags = _plan_fast(pm, tg)
    Rb = _bucket(R)
    run = _get_exec(Rb, Rb <= 112)

    packed = (pm << 4) | tg  # [B, H, W] uint8
    feed = {
        "maskP": packed.reshape(B * H, W),
        "flags": np.ascontiguousarray(
            np.broadcast_to(flags[:, None, :], (B, 128, 6))).reshape(-1, 6),
    }
    outs = run(feed)
    total = float(outs["out"].sum(dtype=np.float64))
    return np.float32(total / (B * H * W))


# revision 4
# speedup vs baseline: 2.1367x; 2.1367x over previous
"""BoundaryLoss kernel for Trainium2 NeuronCores (axon-tunneled).

Computes mean |pred_dist - target_dist| where *_dist are sums of per-class
exact Euclidean distance transforms of the argmax(pred) / target masks.

End-to-end wall time is dominated by the axon dispatch round trip
(~70 ms floor) plus ~12 ms/MB of input transfer, so the design minimizes
host->device bytes and per-call dispatch overhead:

  - argmax(pred) is computed on host; the two class masks are packed into
    ONE uint8 tensor (pred_mask << 4 | target) -- 64 KB per core instead
    of the 1.3 MB of raw fp32 logits.
  - 4 cores, one full image each (no halo rows, both masks on the same
    core so |pred_dist - target_dist| reduces locally to [128,1]).
  - the jitted shard_map dispatcher is built ONCE per (R-bucket) and
    cached; per-call cost is transfer + execute only.

EDT algorithm per (mask, class, image):
  pass 1 (along W): exact nearest-set-pixel row distances via two
    min-plus scans  state = min(state+1, f)  (forward + backward).
  pass 2 (along H): d^2(x) = min_k (dr[x+k]^2 + k^2) windowed to |k| <= R,
    where R is a sound data-derived bound (max row distance plus the max
    empty-row gap), rounded up to a bucket so one NEFF is reused.
"""

import numpy as np

import jax
from jax.sharding import Mesh, PartitionSpec
from jax.experimental.shard_map import shard_map

import concourse.bass as bass
import concourse.bacc as bacc
import concourse.mybir as mybir
from concourse.tile import TileContext
from concourse import bass2jax as _b2j

B, C, H, W = 4, 4, 256, 256
N_CORES = 4
LARGEF = 1.0e6  # pseudo-infinity seed for pass-1 scans (pre-square space)
INF = 1 << 20

F32 = mybir.dt.float32
I16 = mybir.dt.int16
U8 = mybir.dt.uint8
Alu = mybir.AluOpType
Act = mybir.ActivationFunctionType

# R buckets (multiples of 16 keep DMA-transpose offsets 32B-aligned).
# <=112 keeps row dists <= 127 so squares fit comfortably in int16.
_BUCKETS_I16 = (32, 48, 64, 80, 96, 112)
_BUCKETS_F32 = (176, 256, 368)


# ---------------------------------------------------------------- host plan

def _argmax4(pred):
    """First-wins argmax over axis 1 of [B,4,H,W], as uint8."""
    p0, p1, p2, p3 = pred[:, 0], pred[:, 1], pred[:, 2], pred[:, 3]
    m01 = np.maximum(p0, p1)
    m23 = np.maximum(p2, p3)
    i01 = (p1 > p0).astype(np.uint8)
    i23 = np.where(p3 > p2, np.uint8(3), np.uint8(2))
    return np.where(m23 > m01, i23, i01)


def _scan_dists(d):
    """In-place 1D nearest-set distances along axis 0 of an int16 array
    seeded with 0 (set) / 2000 (unset). Row-at-a-time so each numpy call
    is a contiguous SIMD min over all columns; empty columns stay 2000."""
    n = len(d)
    t = np.empty(d.shape[1:], np.int16)
    one = np.int16(1)
    for i in range(1, n):
        np.add(d[i - 1], one, out=t)
        np.minimum(d[i], t, out=d[i])
    for i in range(n - 2, -1, -1):
        np.add(d[i + 1], one, out=t)
        np.minimum(d[i], t, out=d[i])
    return d


def _plan_fast(pm, tg):
    """Window radius R and per-(image, slab) presence flags.

    Row distances are scanned along axis 0 of W-transposed arrays so each
    scan step is one SIMD min across all 6*B*H columns.
    """
    BH = B * H
    pmT = np.ascontiguousarray(pm.reshape(-1, W).T)  # [W, B*H]
    tgT = np.ascontiguousarray(tg.reshape(-1, W).T)
    bsT = np.empty((W, 6 * BH), np.bool_)
    for i, c in enumerate((1, 2, 3)):
        np.equal(pmT, c, out=bsT[:, i * BH:(i + 1) * BH])
        np.equal(tgT, c, out=bsT[:, (3 + i) * BH:(4 + i) * BH])
    d = _scan_dists(np.where(bsT, np.int16(0), np.int16(2000)))
    # row (b,h) of slab s nonempty <=> its dists are real (max real = 255)
    rows_any = (d[0] < 1024).reshape(6, BH)
    r1 = np.where(d < 1024, d, np.int16(-1)).reshape(W, 6, BH).max(axis=(0, 2))

    present = rows_any.reshape(6, B, H).any(axis=2)  # [6, B]
    if rows_any.all():
        vg = np.zeros(6, np.int16)
    else:
        raT = np.ascontiguousarray(rows_any.reshape(6, B, H).reshape(-1, H).T)
        dH = _scan_dists(np.where(raT, np.int16(0), np.int16(2000)))
        vgc = np.where(dH < 1024, dH, np.int16(0)).max(axis=0)  # [6*B]
        vg = np.where(present, vgc.reshape(6, B), 0).max(axis=1)  # [6]

    R = int(max(1, np.minimum(r1 + vg, 361).max()))
    flags = present.T.astype(np.float32)  # [B, 6], slab = mi*3 + (c-1)
    return R, flags


def _bucket(R):
    for b in _BUCKETS_I16 + _BUCKETS_F32:
        if R <= b:
            return b
    return _BUCKETS_F32[-1]


# ---------------------------------------------------------------- device side

def _build(R, use_i16, iters=1):
    P = 2 * R + 256  # padded column length for pass 2
    capv = 127.0 if use_i16 else 400.0
    padv = 30000 if use_i16 else 1.0e9
    DT = I16 if use_i16 else F32

    nc = bacc.Bacc(None, target_bir_lowering=False)
    maskP = nc.dram_tensor("maskP", [H, W], U8, kind="ExternalInput")
    flagsI = nc.dram_tensor("flags", [128, 6], F32, kind="ExternalInput")
    out = nc.dram_tensor("out", [128, 1], F32, kind="ExternalOutput")

    with TileContext(nc) as tc:
        with (
            tc.tile_pool(name="const", bufs=1) as constp,
            tc.tile_pool(name="io", bufs=2) as iop,
            tc.tile_pool(name="p1", bufs=2) as p1p,
            tc.tile_pool(name="h2", bufs=1) as h2p,
            tc.tile_pool(name="fin", bufs=1) as finp,
        ):
            def _body():
                flagst = constp.tile([128, 6], F32)
                nc.gpsimd.dma_start(flagst[:], flagsI[:])
                ones = constp.tile([128, W], F32)
                nc.vector.memset(ones[:], 1.0)

                # W-transposed row-distance maps, 6 slabs
                # (pred c1..c3, targ c1..c3), free len P = 256 + 2R padding.
                # h2d pads (never written by the transposes) hold capv so
                # their squares read as "far" for boundary rows. h2B = h2A
                # shifted one element left (odd window offsets keep the
                # 2x_1P int16 DVE mode).
                h2d = [h2p.tile([128, 6, P], I16, name=f"h2d{w}") for w in range(2)]
                h2A = [h2p.tile([128, 6, P], DT, name=f"h2A{w}") for w in range(2)]
                if use_i16:
                    h2B = [h2p.tile([128, 6, P], DT, name=f"h2B{w}") for w in range(2)]
                accs = [h2p.tile([128, 6, 256], DT, name=f"acc{w}") for w in range(2)]
                for wc in range(2):
                    nc.vector.memset(h2d[wc][:], capv)
                    if use_i16:
                        nc.vector.memset(h2B[wc][:], padv)
                    nc.vector.memset(accs[wc][:], padv)

                # ---------------- pass 1 + transpose, per row-chunk
                for cs in (0, 128):
                    mk = iop.tile([128, W], U8, name="mk")
                    nc.gpsimd.dma_start(mk[:], maskP[cs:cs + 128])
                    pmu = p1p.tile([128, W], U8, name="pmu")
                    nc.vector.tensor_scalar(
                        pmu[:], mk[:], 4, None, op0=Alu.logical_shift_right)
                    tau = p1p.tile([128, W], U8, name="tau")
                    nc.vector.tensor_scalar(
                        tau[:], mk[:], 15, None, op0=Alu.bitwise_and)
                    pmf = p1p.tile([128, W], F32, name="pmf")
                    nc.scalar.activation(pmf[:], pmu[:], Act.Copy)
                    taf = p1p.tile([128, W], F32, name="taf")
                    nc.scalar.activation(taf[:], tau[:], Act.Copy)

                    for slab in range(6):
                        mi, c = divmod(slab, 3)
                        c += 1
                        src = taf if mi == 1 else pmf
                        f = p1p.tile([128, W], F32, name="fseed")
                        nc.vector.tensor_scalar(
                            f[:], src[:], float(c), LARGEF,
                            op0=Alu.not_equal, op1=Alu.mult)
                        a = p1p.tile([128, W], F32, name="a")
                        nc.vector.tensor_tensor_scan(
                            a[:], ones[:], f[:], LARGEF,
                            op0=Alu.add, op1=Alu.min)
                        dd = p1p.tile([128, W], F32, name="dd")
                        nc.vector.tensor_tensor_scan(
                            dd[:, ::-1], ones[:], a[:, ::-1], LARGEF,
                            op0=Alu.add, op1=Alu.min)
                        nc.vector.tensor_scalar_min(dd[:], dd[:], capv)
                        ddi = p1p.tile([128, W], I16, name="ddi")
                        nc.gpsimd.tensor_copy(ddi[:], dd[:])

                        for wc in range(2):
                            nc.sync.dma_start_transpose(
                                h2d[wc][:, slab, R + cs:R + cs + 128],
                                ddi[:, wc * 128:(wc + 1) * 128])

                # squares: h2A = h2d^2, h2B = shifted h2A
                for wc in range(2):
                    nc.scalar.activation(h2A[wc][:], h2d[wc][:], Act.Square)
                    if use_i16:
                        nc.scalar.activation(
                            h2B[wc][:, :, 0:P - 1],
                            h2d[wc][:, :, 1:P], Act.Square)

                # ---------------- pass 2: windowed parabola min-plus along H
                ks = [0]
                for k in range(1, R + 1):
                    ks += [k, -k]
                for k in ks:
                    base = R + k
                    kk = k * k
                    for wc in range(2):
                        if use_i16 and base % 2 == 1:
                            src, b0 = h2B[wc], base - 1
                        else:
                            src, b0 = h2A[wc], base
                        nc.vector.scalar_tensor_tensor(
                            accs[wc][:], src[:, :, b0:b0 + 256],
                            int(kk) if use_i16 else float(kk),
                            accs[wc][:],
                            op0=Alu.add, op1=Alu.min)

                # ---------------- sqrt, class sums, |pred-targ|, reduce
                prt = finp.tile([128, 2], F32)
                for wc in range(2):
                    sq = finp.tile([128, 6, 256], F32, name="sq")
                    for slab in range(6):
                        nc.scalar.activation(
                            sq[:, slab], accs[wc][:, slab], Act.Sqrt)
                        nc.vector.tensor_single_scalar(
                            sq[:, slab], sq[:, slab],
                            flagst[:, slab:slab + 1], op=Alu.mult)
                    sp = finp.tile([128, 256], F32, name="sp")
                    st = finp.tile([128, 256], F32, name="st")
                    nc.vector.tensor_add(sp[:], sq[:, 0], sq[:, 1])
                    nc.vector.tensor_add(sp[:], sp[:], sq[:, 2])
                    nc.vector.tensor_add(st[:], sq[:, 3], sq[:, 4])
                    nc.vector.tensor_add(st[:], st[:], sq[:, 5])
                    nc.vector.tensor_sub(sp[:], sp[:], st[:])
                    nc.vector.tensor_reduce(
                        prt[:, wc:wc + 1], sp[:], axis=mybir.AxisListType.X,
                        op=Alu.add, apply_absolute_value=True)
                total = finp.tile([128, 1], F32)
                nc.vector.tensor_add(total[:], prt[:, 0:1], prt[:, 1:2])
                nc.gpsimd.dma_start(out[:], total[:])

            if iters > 1:
                E = mybir.EngineType
                with tc.For_i(0, iters, 1, hint_engines=(
                        E.DVE, E.Activation, E.Pool, E.SP)):
                    _body()
            else:
                _body()

    nc.finalize()
    return nc


# ---------------------------------------------------------------- dispatcher

_EXEC_CACHE = {}


def _get_exec(Rb, use_i16, iters=1):
    """Build the Bass module + jitted shard_map dispatcher once per bucket."""
    key = (Rb, use_i16, iters)
    if key in _EXEC_CACHE:
        return _EXEC_CACHE[key]

    nc = _build(Rb, use_i16, iters)
    _b2j.install_neuronx_cc_hook()
    assert nc.dbg_addr is None
    part_name = nc.partition_id_tensor.name if nc.partition_id_tensor else None

    in_names, out_names, out_avals, zero_specs = [], [], [], []
    for alloc in nc.m.functions[0].allocations:
        if not isinstance(alloc, mybir.MemoryLocationSet):
            continue
        name = alloc.memorylocations[0].name
        if alloc.kind == "ExternalInput":
            if name != part_name:
                in_names.append(name)
        elif alloc.kind == "ExternalOutput":
            shape = tuple(alloc.tensor_shape)
            dtype = mybir.dt.np(alloc.dtype)
            out_names.append(name)
            out_avals.append(jax.core.ShapedArray(shape, dtype))
            zero_specs.append((shape, dtype))
    n_params = len(in_names)
    all_names = tuple(in_names) + tuple(out_names)
    if part_name is not None:
        all_names += (part_name,)

    def _body(*args):
        operands = list(args)
        if part_name is not None:
            operands.append(_b2j.partition_id_tensor())
        return tuple(_b2j._bass_exec_p.bind(
            *operands,
            out_avals=tuple(out_avals),
            in_names=all_names,
            out_names=tuple(out_names),
            lowering_input_output_aliases=(),
            sim_require_finite=True,
            sim_require_nnan=True,
            nc=nc,
        ))

    mesh = Mesh(np.asarray(jax.devices()[:N_CORES]), ("core",))
    sharded = jax.jit(
        shard_map(
            _body, mesh=mesh,
            in_specs=(PartitionSpec("core"),) * (n_params + len(out_names)),
            out_specs=(PartitionSpec("core"),) * len(out_names),
            check_rep=False,
        ),
        donate_argnums=tuple(range(n_params, n_params + len(out_names))),
        keep_unused=True,
    )

    def run(feed):
        args = [feed[n] for n in in_names]
        zeros = [np.zeros((N_CORES * s[0], *s[1:]), d) for s, d in zero_specs]
        outs = sharded(*args, *zeros)
        return {n: np.asarray(o) for n, o in zip(out_names, outs)}

    _EXEC_CACHE[key] = run
    return run


# ---------------------------------------------------------------- entry point

def kernel(pred, target):
    pred = np.asarray(pred, dtype=np.float32)
    target = np.asarray(target)
    pm = _argmax4(pred)
    tg = target.astype(np.uint8)
    R, flags = _plan_fast(pm, tg)
    Rb = _bucket(R)
    run = _get_exec(Rb, Rb <= 112)

    packed = (pm << 4) | tg  # [B, H, W] uint8
    feed = {
        "maskP": packed.reshape(B * H, W),
        "flags": np.ascontiguousarray(
            np.broadcast_to(flags[:, None, :], (B, 128, 6))).reshape(-1, 6),
    }
    outs = run(feed)
    total = float(outs["out"].sum(dtype=np.float64))
    return np.float32(total / (B * H * W))


# revision 6
# speedup vs baseline: 2.5550x; 1.1958x over previous
"""BoundaryLoss kernel for Trainium2 NeuronCores (axon-tunneled).

Computes mean |pred_dist - target_dist| where *_dist are sums of per-class
exact Euclidean distance transforms of the argmax(pred) / target masks.

End-to-end wall time is dominated by the axon dispatch round trip
(~70 ms floor) plus ~12 ms/MB of input transfer, so the design minimizes
host->device bytes and per-call dispatch overhead:

  - argmax(pred) is computed on host; the two class masks are packed into
    ONE uint8 tensor (pred_mask << 4 | target) -- 64 KB per core instead
    of the 1.3 MB of raw fp32 logits.
  - 4 cores, one full image each (no halo rows, both masks on the same
    core so |pred_dist - target_dist| reduces locally to [128,1]).
  - the jitted shard_map dispatcher is built ONCE per (R-bucket) and
    cached; per-call cost is transfer + execute only.

EDT algorithm per (mask, class, image):
  pass 1 (along W): exact nearest-set-pixel row distances via two
    min-plus scans  state = min(state+1, f)  (forward + backward).
  pass 2 (along H): d^2(x) = min_k (dr[x+k]^2 + k^2) windowed to |k| <= R,
    where R is a sound data-derived bound (max row distance plus the max
    empty-row gap), rounded up to a bucket so one NEFF is reused.
"""

import numpy as np

import jax
from jax.sharding import Mesh, PartitionSpec
from jax.experimental.shard_map import shard_map

import concourse.bass as bass
import concourse.bacc as bacc
import concourse.mybir as mybir
from concourse.tile import TileContext
from concourse import bass2jax as _b2j

B, C, H, W = 4, 4, 256, 256
N_CORES = 4
LARGEF = 1.0e6  # pseudo-infinity seed for pass-1 scans (pre-square space)
INF = 1 << 20

F32 = mybir.dt.float32
I16 = mybir.dt.int16
U8 = mybir.dt.uint8
Alu = mybir.AluOpType
Act = mybir.ActivationFunctionType

# R buckets (multiples of 16 keep DMA-transpose offsets 32B-aligned).
# <=112 keeps row dists <= 127 so squares fit comfortably in int16.
_BUCKETS_I16 = (32, 48, 64, 80, 96, 112)
_BUCKETS_F32 = (176, 256, 368)


# ---------------------------------------------------------------- host plan

def _argmax4(pred):
    """First-wins argmax over axis 1 of [B,4,H,W], as uint8."""
    p0, p1, p2, p3 = pred[:, 0], pred[:, 1], pred[:, 2], pred[:, 3]
    m01 = np.maximum(p0, p1)
    m23 = np.maximum(p2, p3)
    i01 = (p1 > p0).astype(np.uint8)
    i23 = np.where(p3 > p2, np.uint8(3), np.uint8(2))
    return np.where(m23 > m01, i23, i01)


def _scan_dists(d):
    """In-place 1D nearest-set distances along axis 0 of an int16 array
    seeded with 0 (set) / 2000 (unset). Row-at-a-time so each numpy call
    is a contiguous SIMD min over all columns; empty columns stay 2000."""
    n = len(d)
    t = np.empty(d.shape[1:], np.int16)
    one = np.int16(1)
    for i in range(1, n):
        np.add(d[i - 1], one, out=t)
        np.minimum(d[i], t, out=d[i])
    for i in range(n - 2, -1, -1):
        np.add(d[i + 1], one, out=t)
        np.minimum(d[i], t, out=d[i])
    return d


def _plan_fast(pm, tg):
    """Window radius R (sound upper bound) and per-(image, slab) flags.

    Masks are reduced to one byte per 8-pixel row block: bit c set iff the
    block contains class c (via OR-reduce of 1 << mask). Block-level row
    distances db then bound real row distances by 8*db + 7, so
    R = max_slab(r1_bound + max_empty_row_gap) stays sound while touching
    only [W/8, 6*B*H] data in the sequential scans.
    """
    BH = B * H
    NB = W // 8  # row blocks
    bits = np.bitwise_or(
        np.left_shift(np.uint8(1), pm.reshape(BH, NB, 8)),
        np.left_shift(np.uint8(16), tg.reshape(BH, NB, 8)))
    blk = np.bitwise_or.reduce(bits, axis=2)  # [BH, NB] per-block class sets
    blkT = blk.T.copy()  # [NB, BH]
    d = np.empty((NB, 6 * BH), np.int16)
    for i, c in enumerate((1, 2, 3)):
        d[:, i * BH:(i + 1) * BH] = np.where(
            (blkT & np.uint8(1 << c)) != 0, np.int16(0), np.int16(2000))
        d[:, (3 + i) * BH:(4 + i) * BH] = np.where(
            (blkT & np.uint8(16 << c)) != 0, np.int16(0), np.int16(2000))
    _scan_dists(d)
    # row (b,h) of slab s nonempty <=> some block set (dists real, <= NB)
    rows_any = (d[0] < 1024).reshape(6, BH)
    db = np.where(d < 1024, d, np.int16(-1)).reshape(NB, 6, BH).max(axis=(0, 2))
    r1 = 8 * db + 7  # [6]; -1 if slab absent everywhere

    present = rows_any.reshape(6, B, H).any(axis=2)  # [6, B]
    if rows_any.all():
        vg = np.zeros(6, np.int16)
    else:
        raT = np.ascontiguousarray(rows_any.reshape(6, B, H).reshape(-1, H).T)
        dH = _scan_dists(np.where(raT, np.int16(0), np.int16(2000)))
        vgc = np.where(dH < 1024, dH, np.int16(0)).max(axis=0)  # [6*B]
        vg = np.where(present, vgc.reshape(6, B), 0).max(axis=1)  # [6]

    R = int(max(1, np.minimum(r1 + vg, 361).max()))
    flags = present.T.astype(np.float32)  # [B, 6], slab = mi*3 + (c-1)
    return R, flags


def _bucket(R):
    for b in _BUCKETS_I16 + _BUCKETS_F32:
        if R <= b:
            return b
    return _BUCKETS_F32[-1]


# ---------------------------------------------------------------- device side

def _build(R, use_i16, iters=1):
    P = 2 * R + 256  # padded column length for pass 2
    capv = 127.0 if use_i16 else 400.0
    padv = 30000 if use_i16 else 1.0e9
    DT = I16 if use_i16 else F32

    nc = bacc.Bacc(None, target_bir_lowering=False)
    maskP = nc.dram_tensor("maskP", [H, W], U8, kind="ExternalInput")
    flagsI = nc.dram_tensor("flags", [128, 6], F32, kind="ExternalInput")
    out = nc.dram_tensor("out", [128, 1], F32, kind="ExternalOutput")

    with TileContext(nc) as tc:
        with (
            tc.tile_pool(name="const", bufs=1) as constp,
            tc.tile_pool(name="io", bufs=2) as iop,
            tc.tile_pool(name="p1", bufs=2) as p1p,
            tc.tile_pool(name="h2", bufs=1) as h2p,
            tc.tile_pool(name="fin", bufs=1) as finp,
        ):
            def _body():
                flagst = constp.tile([128, 6], F32)
                nc.gpsimd.dma_start(flagst[:], flagsI[:])
                ones = constp.tile([128, W], F32)
                nc.vector.memset(ones[:], 1.0)

                # W-transposed row-distance maps, 6 slabs
                # (pred c1..c3, targ c1..c3), free len P = 256 + 2R padding.
                # h2d pads (never written by the transposes) hold capv so
                # their squares read as "far" for boundary rows. h2B = h2A
                # shifted one element left (odd window offsets keep the
                # 2x_1P int16 DVE mode).
                h2d = [h2p.tile([128, 6, P], I16, name=f"h2d{w}") for w in range(2)]
                h2A = [h2p.tile([128, 6, P], DT, name=f"h2A{w}") for w in range(2)]
                if use_i16:
                    h2B = [h2p.tile([128, 6, P], DT, name=f"h2B{w}") for w in range(2)]
                accs = [h2p.tile([128, 6, 256], DT, name=f"acc{w}") for w in range(2)]
                for wc in range(2):
                    nc.vector.memset(h2d[wc][:], capv)
                    if use_i16:
                        nc.vector.memset(h2B[wc][:], padv)
                    nc.vector.memset(accs[wc][:], padv)

                # ---------------- pass 1 + transpose, per row-chunk
                for cs in (0, 128):
                    mk = iop.tile([128, W], U8, name="mk")
                    nc.gpsimd.dma_start(mk[:], maskP[cs:cs + 128])
                    pmu = p1p.tile([128, W], U8, name="pmu")
                    nc.vector.tensor_scalar(
                        pmu[:], mk[:], 4, None, op0=Alu.logical_shift_right)
                    tau = p1p.tile([128, W], U8, name="tau")
                    nc.vector.tensor_scalar(
                        tau[:], mk[:], 15, None, op0=Alu.bitwise_and)
                    pmf = p1p.tile([128, W], F32, name="pmf")
                    nc.scalar.activation(pmf[:], pmu[:], Act.Copy)
                    taf = p1p.tile([128, W], F32, name="taf")
                    nc.scalar.activation(taf[:], tau[:], Act.Copy)

                    for slab in range(6):
                        mi, c = divmod(slab, 3)
                        c += 1
                        src = taf if mi == 1 else pmf
                        f = p1p.tile([128, W], F32, name="fseed")
                        nc.vector.tensor_scalar(
                            f[:], src[:], float(c), LARGEF,
                            op0=Alu.not_equal, op1=Alu.mult)
                        a = p1p.tile([128, W], F32, name="a")
                        nc.vector.tensor_tensor_scan(
                            a[:], ones[:], f[:], LARGEF,
                            op0=Alu.add, op1=Alu.min)
                        dd = p1p.tile([128, W], F32, name="dd")
                        nc.vector.tensor_tensor_scan(
                            dd[:, ::-1], ones[:], a[:, ::-1], LARGEF,
                            op0=Alu.add, op1=Alu.min)
                        nc.vector.tensor_scalar_min(dd[:], dd[:], capv)
                        ddi = p1p.tile([128, W], I16, name="ddi")
                        nc.gpsimd.tensor_copy(ddi[:], dd[:])

                        for wc in range(2):
                            nc.sync.dma_start_transpose(
                                h2d[wc][:, slab, R + cs:R + cs + 128],
                                ddi[:, wc * 128:(wc + 1) * 128])

                # squares: h2A = h2d^2, h2B = shifted h2A
                for wc in range(2):
                    nc.scalar.activation(h2A[wc][:], h2d[wc][:], Act.Square)
                    if use_i16:
                        nc.scalar.activation(
                            h2B[wc][:, :, 0:P - 1],
                            h2d[wc][:, :, 1:P], Act.Square)

                # ---------------- pass 2: windowed parabola min-plus along H
                ks = [0]
                for k in range(1, R + 1):
                    ks += [k, -k]
                for k in ks:
                    base = R + k
                    kk = k * k
                    for wc in range(2):
                        if use_i16 and base % 2 == 1:
                            src, b0 = h2B[wc], base - 1
                        else:
                            src, b0 = h2A[wc], base
                        nc.vector.scalar_tensor_tensor(
                            accs[wc][:], src[:, :, b0:b0 + 256],
                            int(kk) if use_i16 else float(kk),
                            accs[wc][:],
                            op0=Alu.add, op1=Alu.min)

                # ---------------- sqrt, class sums, |pred-targ|, reduce
                prt = finp.tile([128, 2], F32)
                for wc in range(2):
                    sq = finp.tile([128, 6, 256], F32, name="sq")
                    for slab in range(6):
                        nc.scalar.activation(
                            sq[:, slab], accs[wc][:, slab], Act.Sqrt)
                        nc.vector.tensor_single_scalar(
                            sq[:, slab], sq[:, slab],
                            flagst[:, slab:slab + 1], op=Alu.mult)
                    sp = finp.tile([128, 256], F32, name="sp")
                    st = finp.tile([128, 256], F32, name="st")
                    nc.vector.tensor_add(sp[:], sq[:, 0], sq[:, 1])
                    nc.vector.tensor_add(sp[:], sp[:], sq[:, 2])
                    nc.vector.tensor_add(st[:], sq[:, 3], sq[:, 4])
                    nc.vector.tensor_add(st[:], st[:], sq[:, 5])
                    nc.vector.tensor_sub(sp[:], sp[:], st[:])
                    nc.vector.tensor_reduce(
                        prt[:, wc:wc + 1], sp[:], axis=mybir.AxisListType.X,
                        op=Alu.add, apply_absolute_value=True)
                total = finp.tile([128, 1], F32)
                nc.vector.tensor_add(total[:], prt[:, 0:1], prt[:, 1:2])
                nc.gpsimd.dma_start(out[:], total[:])

            if iters > 1:
                E = mybir.EngineType
                with tc.For_i(0, iters, 1, hint_engines=(
                        E.DVE, E.Activation, E.Pool, E.SP)):
                    _body()
            else:
                _body()

    nc.finalize()
    return nc


# ---------------------------------------------------------------- dispatcher

_EXEC_CACHE = {}


def _get_exec(Rb, use_i16, iters=1):
    """Build the Bass module + jitted shard_map dispatcher once per bucket."""
    key = (Rb, use_i16, iters)
    if key in _EXEC_CACHE:
        return _EXEC_CACHE[key]

    nc = _build(Rb, use_i16, iters)
    _b2j.install_neuronx_cc_hook()
    assert nc.dbg_addr is None
    part_name = nc.partition_id_tensor.name if nc.partition_id_tensor else None

    in_names, out_names, out_avals, zero_specs = [], [], [], []
    for alloc in nc.m.functions[0].allocations:
        if not isinstance(alloc, mybir.MemoryLocationSet):
            continue
        name = alloc.memorylocations[0].name
        if alloc.kind == "ExternalInput":
            if name != part_name:
                in_names.append(name)
        elif alloc.kind == "ExternalOutput":
            shape = tuple(alloc.tensor_shape)
            dtype = mybir.dt.np(alloc.dtype)
            out_names.append(name)
            out_avals.append(jax.core.ShapedArray(shape, dtype))
            zero_specs.append((shape, dtype))
    n_params = len(in_names)
    all_names = tuple(in_names) + tuple(out_names)
    if part_name is not None:
        all_names += (part_name,)

    def _body(*args):
        operands = list(args)
        if part_name is not None:
            operands.append(_b2j.partition_id_tensor())
        return tuple(_b2j._bass_exec_p.bind(
            *operands,
            out_avals=tuple(out_avals),
            in_names=all_names,
            out_names=tuple(out_names),
            lowering_input_output_aliases=(),
            sim_require_finite=True,
            sim_require_nnan=True,
            nc=nc,
        ))

    mesh = Mesh(np.asarray(jax.devices()[:N_CORES]), ("core",))
    sharded = jax.jit(
        shard_map(
            _body, mesh=mesh,
            in_specs=(PartitionSpec("core"),) * (n_params + len(out_names)),
            out_specs=(PartitionSpec("core"),) * len(out_names),
            check_rep=False,
        ),
        donate_argnums=tuple(range(n_params, n_params + len(out_names))),
        keep_unused=True,
    )

    def run(feed):
        args = [feed[n] for n in in_names]
        zeros = [np.zeros((N_CORES * s[0], *s[1:]), d) for s, d in zero_specs]
        outs = sharded(*args, *zeros)
        return {n: np.asarray(o) for n, o in zip(out_names, outs)}

    _EXEC_CACHE[key] = run
    return run


# ---------------------------------------------------------------- entry point

def kernel(pred, target):
    pred = np.asarray(pred, dtype=np.float32)
    target = np.asarray(target)
    pm = _argmax4(pred)
    tg = target.astype(np.uint8)
    R, flags = _plan_fast(pm, tg)
    Rb = _bucket(R)
    run = _get_exec(Rb, Rb <= 112)

    packed = (pm << 4) | tg  # [B, H, W] uint8
    feed = {
        "maskP": packed.reshape(B * H, W),
        "flags": np.ascontiguousarray(
            np.broadcast_to(flags[:, None, :], (B, 128, 6))).reshape(-1, 6),
    }
    outs = run(feed)
    total = float(outs["out"].sum(dtype=np.float64))
    return np.float32(total / (B * H * W))
